# revision 106
# baseline (speedup 1.0000x reference)
"""FAGCN (FAConv x3) Trainium2 kernel, 8-core SPMD, push scheme.

Sharding: nodes partitioned across 8 cores (6250 each, padded to 6272).
Edges assigned to the owner of SRC, so the per-edge gather of
(hs, al) rows reads the core's own local node table -- no big collective
before the edge pass (the old pull design AllGathered a 25.7MB table per
layer, ~49%% of its runtime).  Per layer:

  1. node prep: hs = h*dinv and al = h@att_l written to the local table
     (6272 rows x 512B); ar = h@att_r AllGathered as a tiny [50176]
     bf16 vector, reloaded owner-major as [8, 49, 128] so a K=8
     one-hot matmul (sel8) broadcasts any window's ar row to psar.
  2. edge pass over chunks of 128 edges grouped by global dst window
     (392 windows, half A = local windows 0..24 of every owner first,
     then half B): dma_gather rows by src (<=1024 idxs per call -- the
     SWDGE descriptor ring holds 1024), coef = tanh(al_src + ar_dst)
     via psar + Tanh bias, one-hot matmul accumulates into a psum bank
     holding 4 dst windows, one DVE copy moves the bank to an 8-window
     staging tile, one DMA flushes it to the partition-major partial
     tensor [owner, p, w, c].
  3. ReduceScatter(add) of half A is issued mid-stream (hidden behind
     half B's edge pass); RS of half B at layer end; each core keeps
     its own 6272 rows.  h_new = dinv * acc + EPS * h0.

Edge chunks are padded per (window) group to a multiple of 128
uniformly across cores (SPMD: one program).  Gather indices are local
row ids (< 6272), so a single int16 table with no half-splitting.
Gather calls are prefetched several blocks ahead so the POOL-engine
descriptor generation isn't head-of-line blocked behind collectives.
"""
import os

import numpy as np

import concourse.bacc as bacc
import concourse.bass as bass
import concourse.mybir as mybir
import concourse.tile as tile
from concourse.bass_utils import run_bass_kernel_spmd
from concourse.masks import make_identity

F32 = mybir.dt.float32
BF16 = mybir.dt.bfloat16
I16 = mybir.dt.int16

EPS = 0.1


class Cfg:
    def __init__(self, n_nodes, n_edges, in_dim, out_dim, n_layers,
                 n_cores=8, csup=8, bf16=True):
        self.BF16 = bf16
        self.N = n_nodes
        self.E = n_edges
        self.IN = in_dim
        self.H = 128
        self.OUT = out_dim
        self.NL = n_layers
        self.NC = n_cores
        self.NV = n_nodes // n_cores          # owned nodes per core
        assert self.NV * n_cores == n_nodes
        self.W = (self.NV + 127) // 128       # local windows per core
        self.NP = self.W * 128                # padded nodes per core
        self.NPG = self.NP * n_cores          # padded global nodes
        self.GW = self.W * n_cores            # global dst windows
        self.KT = in_dim // 128               # k-tiles of the input matmul
        assert in_dim % 128 == 0
        self.CSUP = csup                      # chunks per gather call
        self.FB = 7                           # windows per batched DMA flush
        assert self.W % self.FB == 0
        self.WA = (self.W + 1) // 2           # local windows in half A (25)
        self.WB = self.W - self.WA            # local windows in half B (24)


FULL = Cfg(50000, 600000, 512, 64, 3)


# ----------------------------------------------------------------- planner

def plan_edges(cfg, edge_index):
    """Host-side edge sharding (by src owner). Returns the uniform chunk
    schedule and the per-core packed arrays."""
    src = edge_index[0].astype(np.int64)
    dst = edge_index[1].astype(np.int64)
    owner = src // cfg.NV
    sl = src % cfg.NV                                   # local table row
    gd = (dst // cfg.NV) * cfg.NP + (dst % cfg.NV)      # padded global dst
    w = gd >> 7
    rel = gd & 127

    counts = np.zeros((cfg.NC, cfg.GW), np.int64)
    per_core = []
    for c in range(cfg.NC):
        m = owner == c
        s_c, w_c, r_c = sl[m], w[m], rel[m]
        order = np.lexsort((r_c, w_c))
        s_c, w_c, r_c = s_c[order], w_c[order], r_c[order]
        cnt = np.bincount(w_c, minlength=cfg.GW)
        counts[c] = cnt
        per_core.append((s_c, w_c, r_c))

    nch = np.maximum((counts.max(axis=0) + 127) // 128, 1)  # [GW] chunks
    NCH = int(nch.sum())
    EPAD = NCH * 128

    # processing order: half A (wl < WA) for all owners, then half B --
    # lets ReduceScatter of half A overlap half B's edge pass
    worder = []
    for half in range(2):
        for ow in range(cfg.NC):
            for wl in range(cfg.W):
                if (wl >= cfg.WA) == bool(half):
                    worder.append(ow * cfg.W + wl)

    chunk_meta = []   # (window, first_of_group, last_of_group)
    for ww in worder:
        n = int(nch[ww])
        for k in range(n):
            chunk_meta.append((ww, k == 0, k == n - 1))

    starts = np.zeros(cfg.GW, np.int64)
    pos = 0
    for ww in worder:
        starts[ww] = pos
        pos += int(nch[ww]) * 128
    NCHA = int(sum(nch[ww] for ww in worder if (ww % cfg.W) < cfg.WA))

    cores = []
    for c in range(cfg.NC):
        s_c, w_c, r_c = per_core[c]
        gidx = np.zeros(EPAD, np.int64)
        rel_a = np.full(EPAD, 999.0, np.float32)
        ptr = 0
        for ww in range(cfg.GW):
            n = counts[c, ww]
            pos = starts[ww]
            gidx[pos:pos + n] = s_c[ptr:ptr + n]
            rel_a[pos:pos + n] = r_c[ptr:ptr + n].astype(np.float32)
            ptr += n
        assert ptr == len(s_c)

        def wrap16(v):
            a = v.astype(np.int16).reshape(-1, 16).T.copy()
            return np.tile(a, (8, 1))

        def lanes(v):
            return v.reshape(-1, 128).T.copy()

        cores.append(dict(gidx=wrap16(gidx), rel=lanes(rel_a)))
    return dict(nch=nch, NCH=NCH, NCHA=NCHA, EPAD=EPAD,
                chunk_meta=chunk_meta, cores=cores)


def shard_inputs(cfg, inputs, plan):
    """Build per-core in_maps from full inputs."""
    x = np.asarray(inputs["x"], np.float32)
    ei = np.asarray(inputs["edge_index"])
    t1_w = np.asarray(inputs["t1_w"], np.float32)
    t1_b = np.asarray(inputs["t1_b"], np.float32)
    t2_w = np.asarray(inputs["t2_w"], np.float32)
    t2_b = np.asarray(inputs["t2_b"], np.float32)
    att_l = np.asarray(inputs["att_l"], np.float32)
    att_r = np.asarray(inputs["att_r"], np.float32)

    deg_all = np.bincount(ei[1].astype(np.int64), minlength=cfg.N).astype(np.float32)

    w1t = t1_w.T.copy()                      # [IN, H]
    w1t_tiles = w1t.reshape(cfg.KT, 128, cfg.H)
    b1rep = np.broadcast_to(t1_b, (128, cfg.H)).copy()
    alrep = np.stack([np.broadcast_to(att_l[i], (128, cfg.H)) for i in range(cfg.NL)])
    arrep = np.stack([np.broadcast_to(att_r[i], (128, cfg.H)) for i in range(cfg.NL)])
    t2wt = t2_w.T.copy()                     # [H, OUT]
    b2rep = np.broadcast_to(t2_b, (128, cfg.OUT)).copy()
    import ml_dtypes
    bf = ml_dtypes.bfloat16
    iota = np.broadcast_to(np.arange(128, dtype=bf), (128, 128)).copy()
    sel8 = np.zeros((8, 8 * 128), bf)
    for v in range(8):
        sel8[v, v * 128:(v + 1) * 128] = 1.0
    w1t_tiles = w1t_tiles.astype(bf)

    in_maps = []
    for c in range(cfg.NC):
        lo = c * cfg.NV
        xc = np.zeros((cfg.NP, cfg.IN), bf)
        xc[:cfg.NV] = x[lo:lo + cfg.NV]
        xt = xc.reshape(cfg.W, 128, cfg.KT, 128).transpose(0, 3, 2, 1).copy()
        deg = np.zeros(cfg.NP, np.float32)
        deg[:cfg.NV] = deg_all[lo:lo + cfg.NV]
        pc = plan["cores"][c]
        in_maps.append(dict(
            xt=xt, deg=deg,
            w1t=w1t_tiles, b1rep=b1rep, alrep=alrep, arrep=arrep,
            t2wt=t2wt, b2rep=b2rep, iota=iota, sel8=sel8,
            gidx=pc["gidx"], rel=pc["rel"],
        ))
    return in_maps


# ----------------------------------------------------------------- builder

def build_program(cfg, plan, skip=frozenset()):
    NCH = plan["NCH"]
    NCHA = plan["NCHA"]
    meta = plan["chunk_meta"]
    EPAD = plan["EPAD"]
    W = cfg.W
    GW = cfg.GW

    nc = bacc.Bacc("TRN2", target_bir_lowering=False, debug=False,
                   num_devices=cfg.NC, num_swdge_queues=2)

    # ---- I/O
    t_xt = nc.dram_tensor("xt", [W, 128, cfg.KT, 128], BF16, kind="ExternalInput")
    t_deg = nc.dram_tensor("deg", [cfg.NP], F32, kind="ExternalInput")
    t_w1t = nc.dram_tensor("w1t", [cfg.KT, 128, cfg.H], BF16, kind="ExternalInput")
    t_b1 = nc.dram_tensor("b1rep", [128, cfg.H], F32, kind="ExternalInput")
    t_al = nc.dram_tensor("alrep", [cfg.NL, 128, cfg.H], F32, kind="ExternalInput")
    t_ar = nc.dram_tensor("arrep", [cfg.NL, 128, cfg.H], F32, kind="ExternalInput")
    t_t2 = nc.dram_tensor("t2wt", [cfg.H, cfg.OUT], F32, kind="ExternalInput")
    t_b2 = nc.dram_tensor("b2rep", [128, cfg.OUT], F32, kind="ExternalInput")
    t_iota = nc.dram_tensor("iota", [128, 128], BF16, kind="ExternalInput")
    t_sel8 = nc.dram_tensor("sel8", [8, 8 * 128], BF16, kind="ExternalInput")
    t_gidx = nc.dram_tensor("gidx", [128, EPAD // 16], I16, kind="ExternalInput")
    t_rel = nc.dram_tensor("rel", [128, NCH], F32, kind="ExternalInput")
    t_lsm = nc.dram_tensor("lsm", [cfg.NP, cfg.OUT], F32, kind="ExternalOutput")
    t_emb = nc.dram_tensor("emb", [cfg.NP, cfg.OUT], F32, kind="ExternalOutput")

    # ---- internal DRAM
    TDT = BF16 if cfg.BF16 else F32
    RWE = 256 if cfg.BF16 else 192          # table row elems (512B / 768B)
    d_tab = nc.dram_tensor("tab", [cfg.NP, RWE], TDT)
    d_ar_loc = nc.dram_tensor("ar_loc", [cfg.NP], BF16)
    d_ar_full = nc.dram_tensor("ar_full", [cfg.NPG], BF16, addr_space="Shared")
    # partial sums, partition-major per owner segment: [owner, p, w, c];
    # split into window-halves so RS(A) overlaps half B's edge pass
    d_accA = nc.dram_tensor("accA", [cfg.NC, 128, cfg.WA, cfg.H], BF16)
    d_accB = nc.dram_tensor("accB", [cfg.NC, 128, cfg.WB, cfg.H], BF16)
    d_acc_outA = nc.dram_tensor("acc_outA", [128, cfg.WA, cfg.H], BF16)
    d_acc_outB = nc.dram_tensor("acc_outB", [128, cfg.WB, cfg.H], BF16)

    CS = cfg.CSUP
    rg = [list(range(cfg.NC))]

    with tile.TileContext(nc) as tc:
        with (
            tc.tile_pool(name="const", bufs=1) as cp,
            tc.tile_pool(name="stage", bufs=3) as sp,
            tc.tile_pool(name="gath", bufs=8) as gp,
            tc.tile_pool(name="xload", bufs=2) as xp,
            tc.tile_pool(name="oh", bufs=8) as op,
            tc.tile_pool(name="fl", bufs=4) as fp,
            tc.tile_pool(name="stg2", bufs=2) as s2,
            tc.tile_pool(name="small", bufs=4) as mp,
            tc.tile_pool(name="psum", bufs=2, space="PSUM") as pp,
            tc.tile_pool(name="psbk", bufs=3, space="PSUM") as pb_pool,
            tc.tile_pool(name="psar", bufs=3, space="PSUM") as pa,
        ):
            # ---------- constants / persistent state
            w1 = cp.tile([128, cfg.KT, cfg.H], BF16, tag="w1")
            nc.sync.dma_start(out=w1[:], in_=t_w1t[:].rearrange("k p h -> p k h"))
            b1 = cp.tile([128, cfg.H], F32, tag="b1")
            nc.sync.dma_start(out=b1[:], in_=t_b1[:])
            alr = cp.tile([128, cfg.NL, cfg.H], F32, tag="alr")
            nc.sync.dma_start(out=alr[:], in_=t_al[:].rearrange("l p h -> p l h"))
            arr = cp.tile([128, cfg.NL, cfg.H], F32, tag="arr")
            nc.sync.dma_start(out=arr[:], in_=t_ar[:].rearrange("l p h -> p l h"))
            t2w = cp.tile([cfg.H, cfg.OUT], F32, tag="t2w")
            nc.sync.dma_start(out=t2w[:], in_=t_t2[:])
            b2 = cp.tile([128, cfg.OUT], F32, tag="b2")
            nc.sync.dma_start(out=b2[:], in_=t_b2[:])
            iota = cp.tile([128, 128], BF16, tag="iota")
            nc.sync.dma_start(out=iota[:], in_=t_iota[:])
            gidx = cp.tile([128, EPAD // 16], I16, tag="gidx")
            nc.sync.dma_start(out=gidx[:], in_=t_gidx[:])
            sel8 = cp.tile([8, 8 * 128], BF16, tag="sel8")
            nc.sync.dma_start(out=sel8[:], in_=t_sel8[:])
            rel = cp.tile([128, NCH], F32, tag="rel")
            nc.sync.dma_start(out=rel[:], in_=t_rel[:])
            ident = cp.tile([128, 128], F32, tag="ident")
            make_identity(nc, ident[:])

            h_sb = cp.tile([128, W, cfg.H], F32, tag="h")
            raw_sb = cp.tile([128, W, cfg.H], BF16, tag="raw")
            accl = cp.tile([128, W, cfg.H], BF16, tag="accl")
            dinv = cp.tile([128, W], F32, tag="dinv")
            alc = cp.tile([128, W], F32, tag="alc")
            arc = cp.tile([128, W], F32, tag="arc")
            ar8 = cp.tile([8, W, 128], BF16, tag="ar8")

            # ---------- dinv = (deg>0) / sqrt(max(deg,1))
            degt = mp.tile([128, W], F32, tag="degt")
            with nc.allow_non_contiguous_dma(reason="node-col load"):
                nc.sync.dma_start(out=degt[:], in_=t_deg[:].rearrange("(t p) -> p t", p=128))
            dmax = mp.tile([128, W], F32, tag="dmax")
            nc.vector.tensor_scalar_max(dmax[:], degt[:], 1.0)
            nc.scalar.sqrt(dmax[:], dmax[:])
            nc.vector.reciprocal(dmax[:], dmax[:])
            dnz = mp.tile([128, W], F32, tag="dnz")
            nc.vector.tensor_scalar(dnz[:], degt[:], 0.0, None,
                                    op0=mybir.AluOpType.is_gt)
            nc.vector.tensor_tensor(out=dinv[:], in0=dmax[:], in1=dnz[:],
                                    op=mybir.AluOpType.mult)

            # ---------- phase A: h = relu(x @ t1_w.T + b1)
            AB = 7   # node-tiles per x load
            for t0 in range(0, W if "phasea" not in skip else 0, AB):
                nb = min(AB, W - t0)
                xa = xp.tile([128, AB * cfg.KT * 128], BF16, tag="xa")
                nc.sync.dma_start(
                    out=xa[:, :nb * cfg.KT * 128],
                    in_=t_xt[t0:t0 + nb].rearrange("w p k n -> p w k n"))
                for ti in range(nb):
                    t = t0 + ti
                    ps = pp.tile([128, cfg.H], F32, tag="ps")
                    for k in range(cfg.KT):
                        o = (ti * cfg.KT + k) * 128
                        nc.tensor.matmul(ps[:], lhsT=xa[:, o:o + 128],
                                         rhs=w1[:, k, :],
                                         start=(k == 0), stop=(k == cfg.KT - 1))
                    hb = sp.tile([128, cfg.H], F32, tag="hb")
                    nc.vector.tensor_add(hb[:], ps[:], b1[:])
                    nc.scalar.activation(h_sb[:, t, :], hb[:],
                                         mybir.ActivationFunctionType.Relu)
                    nc.gpsimd.tensor_scalar_mul(raw_sb[:, t, :], h_sb[:, t, :],
                                                EPS)

            # ---------- layers
            FB = cfg.FB
            FL8 = 8   # windows per acc flush DMA (2 psum banks)

            # phase C per-window emission (used early for half A during the
            # last layer's half-B edge pass, late for the rest)
            shs = cp.tile([128, W, cfg.OUT], F32, tag="shs")
            sms = cp.tile([128, W], F32, tag="sms")
            pcst = {"estg": None, "e0": 0}
            pc_done = [False] * W

            def phasec_win(t):
                pc_done[t] = True
                pst = pp.tile([128, 128], F32, tag="ps")
                nc.tensor.transpose(out=pst[:], in_=h_sb[:, t, :],
                                    identity=ident[:])
                ht = sp.tile([128, 128], F32, tag="ht")
                nc.vector.tensor_copy(ht[:], pst[:])
                pse = pp.tile([128, cfg.OUT], F32, tag="ps")
                nc.tensor.matmul(pse[:], lhsT=ht[:], rhs=t2w[:],
                                 start=True, stop=True)
                if pcst["estg"] is None:
                    estg = s2.tile([128, FB, cfg.OUT], F32, tag="estg")
                    pcst["estg"] = estg
                    pcst["e0"] = t
                off = t - pcst["e0"]
                emb = pcst["estg"][:, off, :]
                nc.vector.tensor_add(emb, pse[:], b2[:])
                mx = mp.tile([128, 1], F32, tag="mx")
                nc.vector.tensor_reduce(mx[:], emb, axis=mybir.AxisListType.X,
                                        op=mybir.AluOpType.max)
                nc.vector.tensor_scalar(shs[:, t, :], emb, mx[:], None,
                                        op0=mybir.AluOpType.subtract)
                if off == FB - 1 or t in (cfg.WA - 1, W - 1):
                    nc.sync.dma_start(
                        out=t_emb[(t - off) * 128:(t + 1) * 128, :].rearrange(
                            "(t p) c -> p t c", p=128),
                        in_=pcst["estg"][:, :off + 1, :])
                    pcst["estg"] = None
            for li in range(cfg.NL):
                # node-side: al, ar, hs -> local table
                hstg = None
                for t in range(W if "nprep" not in skip else 0):
                    tmp = sp.tile([128, cfg.H], F32, tag="nprep")
                    nc.vector.scalar_tensor_tensor(
                        tmp[:], h_sb[:, t, :], 1.0, alr[:, li, :],
                        op0=mybir.AluOpType.mult, op1=mybir.AluOpType.mult,
                        accum_out=alc[:, t:t + 1])
                    tmp2 = sp.tile([128, cfg.H], F32, tag="nprep2")
                    nc.vector.scalar_tensor_tensor(
                        tmp2[:], h_sb[:, t, :], 1.0, arr[:, li, :],
                        op0=mybir.AluOpType.mult, op1=mybir.AluOpType.mult,
                        accum_out=arc[:, t:t + 1])
                    if t % FB == 0:
                        hstg = s2.tile([128, FB, cfg.H], TDT, tag="hstg")
                    nc.gpsimd.tensor_scalar_mul(hstg[:, t % FB, :], h_sb[:, t, :],
                                                dinv[:, t:t + 1])
                    if t % FB == FB - 1:
                        t0 = t - FB + 1
                        nc.sync.dma_start(
                            out=d_tab[t0 * 128:(t + 1) * 128, :cfg.H].rearrange(
                                "(t p) c -> p t c", p=128),
                            in_=hstg[:])
                alx = sp.tile([128, W], TDT, tag="alx")
                nc.vector.tensor_copy(alx[:], alc[:])
                with nc.allow_non_contiguous_dma(reason="node-col store"):
                    nc.sync.dma_start(
                        out=d_tab[:, cfg.H:cfg.H + 1].rearrange(
                            "(t p) c -> p (t c)", p=128),
                        in_=alx[:])
                arcb = sp.tile([128, W], BF16, tag="arcb")
                nc.gpsimd.tensor_copy(arcb[:], arc[:])
                with nc.allow_non_contiguous_dma(reason="ar-col store"):
                    nc.sync.dma_start(out=d_ar_loc[:].rearrange("(t p) -> p t", p=128),
                                      in_=arcb[:])
                # edge pass; windows stream in global order, flushed FB at
                # a time into the partition-major partial-sum tensor.  The
                # ar AllGather and RS(A) are emitted a few gather blocks
                # late so their POOL-queue occupancy hides behind gathers
                # that are already in flight.
                psw = None
                psar = None
                fl = None
                rsa_emit = NCHA + 6 * CS      # emit RS(A) after this chunk
                hna_emit = min(NCHA + 20 * CS, NCH - 1)
                nblk = (NCH + CS - 1) // CS
                gtiles = [None] * nblk
                issued = 0
                PF = int(os.environ.get("PF", "7"))  # gather-call prefetch depth

                def issue_gather(k):
                    c0 = k * CS
                    nch_call = min(CS, NCH - c0)
                    ne = nch_call * 128
                    ghs = gp.tile([128, CS * RWE], TDT, tag="ghs")
                    gtiles[k] = ghs
                    if "gather" in skip:
                        nc.vector.memset(ghs[:], 0.0)
                    else:
                        nc.gpsimd.dma_gather(
                            out_ap=ghs[:, :nch_call * RWE].rearrange(
                                "p (c e) -> p c e", e=RWE),
                            in_ap=d_tab[:], idxs_ap=gidx[:, c0 * 8:(c0 + nch_call) * 8],
                            num_idxs=ne, num_idxs_reg=ne, elem_size=RWE,
                            queue_num=0)

                for blk in range(nblk):
                    while issued <= min(blk + PF, nblk - 1):
                        issue_gather(issued)
                        issued += 1
                        if issued == 1:
                            # ar AllGather; owner-major [8, wl, 128] for
                            # SEL8.  Emitted after the first gather call so
                            # that call's DGE isn't queued behind the
                            # collective on POOL.
                            if "ag" not in skip:
                                nc.gpsimd.collective_compute(
                                    "AllGather", mybir.AluOpType.bypass,
                                    replica_groups=rg,
                                    ins=[d_ar_loc[:]], outs=[d_ar_full[:]])
                            nc.sync.dma_start(
                                out=ar8[:],
                                in_=d_ar_full[:].rearrange("(p s n) -> p s n",
                                                           p=8, n=128))
                    c0 = blk * CS
                    nch_call = min(CS, NCH - c0)
                    ghs = gtiles[blk]
                    for j in range(nch_call if "chunk" not in skip else 0):
                        ci = c0 + j
                        ww, first, last = meta[ci]
                        ow_, wl_ = divmod(ww, W)
                        inB = wl_ >= cfg.WA
                        wb_ = wl_ - cfg.WA if inB else wl_
                        nw_ = cfg.WB if inB else cfg.WA
                        sl4 = wb_ % 4
                        if first:
                            # ar_rep[p, n] = ar[window ww, node n]: K=8
                            # one-hot owner-selection matmul from ar8
                            psar = pa.tile([128, 128], F32, tag="psar")
                            nc.tensor.matmul(
                                psar[:], lhsT=sel8[:, ow_ * 128:(ow_ + 1) * 128],
                                rhs=ar8[:, wl_, :],
                                start=True, stop=True)
                            if sl4 == 0:
                                # a psum bank accumulates 4 dst windows
                                pbk = pb_pool.tile([128, 512], F32, tag="pbk")
                            psw = pbk[:, sl4 * 128:(sl4 + 1) * 128]
                        tt = op.tile([128, 128], BF16, tag="tt")
                        nc.scalar.activation(
                            tt[:], psar[:], mybir.ActivationFunctionType.Tanh,
                            bias=ghs[:, j * RWE + cfg.H:j * RWE + cfg.H + 1])
                        ohp = op.tile([128, 128], TDT, tag="ohp")
                        nc.vector.scalar_tensor_tensor(
                            ohp[:], iota[:], rel[:, ci:ci + 1], tt[:],
                            op0=mybir.AluOpType.is_equal,
                            op1=mybir.AluOpType.mult)
                        nc.tensor.matmul(psw, lhsT=ohp[:],
                                         rhs=ghs[:, j * RWE:j * RWE + cfg.H],
                                         start=first, stop=last,
                                         skip_group_check=True)
                        if last:
                            if wb_ % FL8 == 0:
                                fl = fp.tile([128, FL8 * cfg.H], BF16, tag="fl")
                            if sl4 == 3 or wb_ == nw_ - 1:
                                # bank done: one DVE copy moves 4 windows
                                b4 = (wb_ % FL8) - sl4
                                nc.vector.tensor_copy(
                                    fl[:, b4 * cfg.H:(b4 + sl4 + 1) * cfg.H],
                                    pbk[:, :(sl4 + 1) * 128])
                            if wb_ % FL8 == FL8 - 1 or wb_ == nw_ - 1:
                                f0 = wb_ - (wb_ % FL8)
                                n_ = wb_ % FL8 + 1
                                d_ = d_accB if inB else d_accA
                                nc.sync.dma_start(
                                    out=d_[ow_, :, f0:wb_ + 1, :],
                                    in_=fl[:, :n_ * cfg.H])
                        if ci == rsa_emit and "rs" not in skip:
                            # half A was flushed several blocks ago:
                            # reduce-scatter it, hidden behind half B's
                            # in-flight gathers
                            nc.gpsimd.collective_compute(
                                "ReduceScatter", mybir.AluOpType.add,
                                replica_groups=rg,
                                ins=[d_accA[:]], outs=[d_acc_outA[:]])
                            nc.sync.dma_start(out=accl[:, :cfg.WA, :],
                                              in_=d_acc_outA[:])
                        if ci == hna_emit and "rs" not in skip:
                            for t in range(cfg.WA if "nprep" not in skip else 0):
                                nc.vector.scalar_tensor_tensor(
                                    h_sb[:, t, :], accl[:, t, :],
                                    dinv[:, t:t + 1], raw_sb[:, t, :],
                                    op0=mybir.AluOpType.mult,
                                    op1=mybir.AluOpType.add)


                # combine half-B partials; keep own rows
                if "rs" not in skip:
                    nc.gpsimd.collective_compute(
                        "ReduceScatter", mybir.AluOpType.add, replica_groups=rg,
                        ins=[d_accB[:]], outs=[d_acc_outB[:]])
                nc.sync.dma_start(out=accl[:, cfg.WA:, :], in_=d_acc_outB[:])
                # h_new = dinv * acc + raw_eps   (raw_eps = EPS*h0)
                for t in range(cfg.WA, W) if "nprep" not in skip else []:
                    nc.vector.scalar_tensor_tensor(
                        h_sb[:, t, :], accl[:, t, :],
                        dinv[:, t:t + 1], raw_sb[:, t, :],
                        op0=mybir.AluOpType.mult, op1=mybir.AluOpType.add)

            # ---------- phase C tail: remaining windows, then log_softmax
            for t in range(W if "phasec" not in skip else 0):
                if not pc_done[t]:
                    phasec_win(t)
            for t in range(W if "phasec" not in skip else 0):
                ex = sp.tile([128, cfg.OUT], F32, tag="ex")
                nc.scalar.activation(ex[:], shs[:, t, :],
                                     mybir.ActivationFunctionType.Exp)
                nc.vector.tensor_reduce(sms[:, t:t + 1], ex[:],
                                        axis=mybir.AxisListType.X,
                                        op=mybir.AluOpType.add)
            if "phasec" not in skip:
                lns = mp.tile([128, W], F32, tag="lns")
                nc.scalar.activation(lns[:], sms[:],
                                     mybir.ActivationFunctionType.Ln)
                lstg = None
                for t in range(W):
                    if t % FB == 0:
                        lstg = s2.tile([128, FB, cfg.OUT], F32, tag="lstg")
                    nc.vector.tensor_scalar(lstg[:, t % FB, :], shs[:, t, :],
                                            lns[:, t:t + 1],
                                            None, op0=mybir.AluOpType.subtract)
                    if t % FB == FB - 1:
                        nc.sync.dma_start(
                            out=t_lsm[(t - FB + 1) * 128:(t + 1) * 128, :].rearrange(
                                "(t p) c -> p t c", p=128),
                            in_=lstg[:])

    nc.finalize()
    return nc


# ------------------------------------------------------- cached PJRT runner

def _make_runner(nc, n_cores):
    """Like bass2jax.run_bass_via_pjrt, but builds the jitted executable once
    so repeated calls don't re-trace/re-compile."""
    import jax
    import concourse.mybir as mb
    from jax.sharding import Mesh, PartitionSpec
    from jax.experimental.shard_map import shard_map
    from concourse.bass2jax import (install_neuronx_cc_hook, partition_id_tensor,
                                    _bass_exec_p)
    install_neuronx_cc_hook()
    partition_name = nc.partition_id_tensor.name if nc.partition_id_tensor else None
    in_names, out_names, out_avals, zero_outs = [], [], [], []
    for alloc in nc.m.functions[0].allocations:
        if not isinstance(alloc, mb.MemoryLocationSet):
            continue
        name = alloc.memorylocations[0].name
        if alloc.kind == "ExternalInput":
            if name != partition_name:
                in_names.append(name)
        elif alloc.kind == "ExternalOutput":
            out_names.append(name)
            shape = tuple(alloc.tensor_shape)
            dtype = mb.dt.np(alloc.dtype)
            out_avals.append(jax.core.ShapedArray(shape, dtype))
            zero_outs.append(np.zeros(shape, dtype))
    n_params = len(in_names)
    n_outs = len(out_avals)
    all_in_names = list(in_names) + list(out_names)
    if partition_name is not None:
        all_in_names.append(partition_name)
    donate = tuple(range(n_params, n_params + n_outs))

    def _body(*args):
        operands = list(args)
        if partition_name is not None:
            operands.append(partition_id_tensor())
        return tuple(_bass_exec_p.bind(
            *operands, out_avals=tuple(out_avals), in_names=tuple(all_in_names),
            out_names=tuple(out_names), lowering_input_output_aliases=(),
            sim_require_finite=True, sim_require_nnan=True, nc=nc))

    devices = jax.devices()[:n_cores]
    mesh = Mesh(np.asarray(devices), ("core",))
    in_specs = (PartitionSpec("core"),) * (n_params + n_outs)
    out_specs = (PartitionSpec("core"),) * n_outs
    sharded = jax.jit(
        shard_map(_body, mesh=mesh, in_specs=in_specs, out_specs=out_specs,
                  check_rep=False),
        donate_argnums=donate, keep_unused=True)

    def call(in_maps):
        concat_in = [
            np.concatenate([np.asarray(in_maps[c][k]) for c in range(n_cores)], 0)
            for k in in_names
        ]
        concat_zeros = [
            np.zeros((n_cores * z.shape[0], *z.shape[1:]), z.dtype)
            for z in zero_outs
        ]
        out_arrs = sharded(*concat_in, *concat_zeros)
        jax.block_until_ready(out_arrs)
        return [
            {k: np.asarray(out_arrs[i]).reshape(n_cores, *out_avals[i].shape)[c]
             for i, k in enumerate(out_names)}
            for c in range(n_cores)
        ]

    return call


# The previous pull-based kernel measured 2127000 ns on hardware (layer
# differencing) and simulates at 1738.6 us under the CoreSim cost model;
# this push kernel simulates at 1123.9 us.  Scaling the hardware estimate
# by the simulated ratio: 2127000 * 1123.9/1738.6.
HW_EXEC_NS_ESTIMATE = 1375000

# ----------------------------------------------------------------- entry

_CACHE = {}


def run(cfg, inputs, trace=False):
    ei = np.asarray(inputs["edge_index"])
    key = (cfg.N, cfg.E, cfg.NL, hash(ei.tobytes()))
    if key in _CACHE:
        runner, plan = _CACHE[key]
    else:
        plan = plan_edges(cfg, ei)
        nc = build_program(cfg, plan)
        runner = _make_runner(nc, cfg.NC)
        _CACHE[key] = (runner, plan)
    in_maps = shard_inputs(cfg, inputs, plan)
    results = runner(in_maps)
    lsm = np.concatenate([results[c]["lsm"][:cfg.NV] for c in range(cfg.NC)], 0)
    emb = np.concatenate([results[c]["emb"][:cfg.NV] for c in range(cfg.NC)], 0)
    return (lsm, emb), None


def kernel(**inputs):
    (lsm, emb), _ = run(FULL, inputs)
    return lsm, emb


# revision 110
# speedup vs baseline: 1.0362x; 1.0362x over previous
"""FAGCN (FAConv x3) Trainium2 kernel, 8-core SPMD, push scheme.

Sharding: nodes partitioned across 8 cores (6250 each, padded to 6272).
Edges assigned to the owner of SRC, so the per-edge gather of
(hs, al) rows reads the core's own local node table -- no big collective
before the edge pass (the old pull design AllGathered a 25.7MB table per
layer, ~49%% of its runtime).  Per layer:

  1. node prep: hs = h*dinv and al = h@att_l written to the local table
     (6272 rows x 512B); ar = h@att_r AllGathered as a tiny [50176]
     bf16 vector, reloaded owner-major as [8, 49, 128] so a K=8
     one-hot matmul (sel8) broadcasts any window's ar row to psar.
  2. edge pass over chunks of 128 edges grouped by global dst window
     (392 windows, half A = local windows 0..24 of every owner first,
     then half B): dma_gather rows by src (<=1024 idxs per call -- the
     SWDGE descriptor ring holds 1024), coef = tanh(al_src + ar_dst)
     via psar + Tanh bias, one-hot matmul accumulates into a psum bank
     holding 4 dst windows, one DVE copy moves the bank to an 8-window
     staging tile, one DMA flushes it to the partition-major partial
     tensor [owner, p, w, c].
  3. ReduceScatter(add) of half A is issued mid-stream (hidden behind
     half B's edge pass); RS of half B at layer end; each core keeps
     its own 6272 rows.  h_new = dinv * acc + EPS * h0.

Edge chunks are padded per (window) group to a multiple of 128
uniformly across cores (SPMD: one program).  Gather indices are local
row ids (< 6272), so a single int16 table with no half-splitting.
Gather calls are prefetched several blocks ahead so the POOL-engine
descriptor generation isn't head-of-line blocked behind collectives.
"""
import os

import numpy as np

import concourse.bacc as bacc
import concourse.bass as bass
import concourse.mybir as mybir
import concourse.tile as tile
from concourse.bass_utils import run_bass_kernel_spmd
from concourse.masks import make_identity

F32 = mybir.dt.float32
BF16 = mybir.dt.bfloat16
I16 = mybir.dt.int16

EPS = 0.1


class Cfg:
    def __init__(self, n_nodes, n_edges, in_dim, out_dim, n_layers,
                 n_cores=8, csup=8, bf16=True):
        self.BF16 = bf16
        self.N = n_nodes
        self.E = n_edges
        self.IN = in_dim
        self.H = 128
        self.OUT = out_dim
        self.NL = n_layers
        self.NC = n_cores
        self.NV = n_nodes // n_cores          # owned nodes per core
        assert self.NV * n_cores == n_nodes
        self.W = (self.NV + 127) // 128       # local windows per core
        self.NP = self.W * 128                # padded nodes per core
        self.NPG = self.NP * n_cores          # padded global nodes
        self.GW = self.W * n_cores            # global dst windows
        self.KT = in_dim // 128               # k-tiles of the input matmul
        assert in_dim % 128 == 0
        self.CSUP = csup                      # chunks per gather call
        self.FB = 7                           # windows per batched DMA flush
        assert self.W % self.FB == 0
        self.WA = 36                          # local windows in half A
        self.WB = self.W - self.WA            # local windows in half B (24)


FULL = Cfg(50000, 600000, 512, 64, 3)


# ----------------------------------------------------------------- planner

def plan_edges(cfg, edge_index):
    """Host-side edge sharding (by src owner). Returns the uniform chunk
    schedule and the per-core packed arrays."""
    src = edge_index[0].astype(np.int64)
    dst = edge_index[1].astype(np.int64)
    owner = src // cfg.NV
    sl = src % cfg.NV                                   # local table row
    gd = (dst // cfg.NV) * cfg.NP + (dst % cfg.NV)      # padded global dst
    w = gd >> 7
    rel = gd & 127

    counts = np.zeros((cfg.NC, cfg.GW), np.int64)
    per_core = []
    for c in range(cfg.NC):
        m = owner == c
        s_c, w_c, r_c = sl[m], w[m], rel[m]
        order = np.lexsort((r_c, w_c))
        s_c, w_c, r_c = s_c[order], w_c[order], r_c[order]
        cnt = np.bincount(w_c, minlength=cfg.GW)
        counts[c] = cnt
        per_core.append((s_c, w_c, r_c))

    nch = np.maximum((counts.max(axis=0) + 127) // 128, 1)  # [GW] chunks
    NCH = int(nch.sum())
    EPAD = NCH * 128

    # processing order: half A (wl < WA) for all owners, then half B --
    # lets ReduceScatter of half A overlap half B's edge pass
    worder = []
    for half in range(2):
        for ow in range(cfg.NC):
            for wl in range(cfg.W):
                if (wl >= cfg.WA) == bool(half):
                    worder.append(ow * cfg.W + wl)

    chunk_meta = []   # (window, first_of_group, last_of_group)
    for ww in worder:
        n = int(nch[ww])
        for k in range(n):
            chunk_meta.append((ww, k == 0, k == n - 1))

    starts = np.zeros(cfg.GW, np.int64)
    pos = 0
    for ww in worder:
        starts[ww] = pos
        pos += int(nch[ww]) * 128
    NCHA = int(sum(nch[ww] for ww in worder if (ww % cfg.W) < cfg.WA))

    cores = []
    for c in range(cfg.NC):
        s_c, w_c, r_c = per_core[c]
        gidx = np.zeros(EPAD, np.int64)
        rel_a = np.full(EPAD, 999.0, np.float32)
        ptr = 0
        for ww in range(cfg.GW):
            n = counts[c, ww]
            pos = starts[ww]
            gidx[pos:pos + n] = s_c[ptr:ptr + n]
            rel_a[pos:pos + n] = r_c[ptr:ptr + n].astype(np.float32)
            ptr += n
        assert ptr == len(s_c)

        def wrap16(v):
            a = v.astype(np.int16).reshape(-1, 16).T.copy()
            return np.tile(a, (8, 1))

        def lanes(v):
            return v.reshape(-1, 128).T.copy()

        cores.append(dict(gidx=wrap16(gidx), rel=lanes(rel_a)))
    return dict(nch=nch, NCH=NCH, NCHA=NCHA, EPAD=EPAD,
                chunk_meta=chunk_meta, cores=cores)


def shard_inputs(cfg, inputs, plan):
    """Build per-core in_maps from full inputs."""
    x = np.asarray(inputs["x"], np.float32)
    ei = np.asarray(inputs["edge_index"])
    t1_w = np.asarray(inputs["t1_w"], np.float32)
    t1_b = np.asarray(inputs["t1_b"], np.float32)
    t2_w = np.asarray(inputs["t2_w"], np.float32)
    t2_b = np.asarray(inputs["t2_b"], np.float32)
    att_l = np.asarray(inputs["att_l"], np.float32)
    att_r = np.asarray(inputs["att_r"], np.float32)

    deg_all = np.bincount(ei[1].astype(np.int64), minlength=cfg.N).astype(np.float32)

    w1t = t1_w.T.copy()                      # [IN, H]
    w1t_tiles = w1t.reshape(cfg.KT, 128, cfg.H)
    b1rep = np.broadcast_to(t1_b, (128, cfg.H)).copy()
    alrep = np.stack([np.broadcast_to(att_l[i], (128, cfg.H)) for i in range(cfg.NL)])
    arrep = np.stack([np.broadcast_to(att_r[i], (128, cfg.H)) for i in range(cfg.NL)])
    t2wt = t2_w.T.copy()                     # [H, OUT]
    b2rep = np.broadcast_to(t2_b, (128, cfg.OUT)).copy()
    import ml_dtypes
    bf = ml_dtypes.bfloat16
    iota = np.broadcast_to(np.arange(128, dtype=bf), (128, 128)).copy()
    sel8 = np.zeros((8, 8 * 128), bf)
    for v in range(8):
        sel8[v, v * 128:(v + 1) * 128] = 1.0
    w1t_tiles = w1t_tiles.astype(bf)

    in_maps = []
    for c in range(cfg.NC):
        lo = c * cfg.NV
        xc = np.zeros((cfg.NP, cfg.IN), bf)
        xc[:cfg.NV] = x[lo:lo + cfg.NV]
        xt = xc.reshape(cfg.W, 128, cfg.KT, 128).transpose(0, 3, 2, 1).copy()
        deg = np.zeros(cfg.NP, np.float32)
        deg[:cfg.NV] = deg_all[lo:lo + cfg.NV]
        pc = plan["cores"][c]
        in_maps.append(dict(
            xt=xt, deg=deg,
            w1t=w1t_tiles, b1rep=b1rep, alrep=alrep, arrep=arrep,
            t2wt=t2wt, b2rep=b2rep, iota=iota, sel8=sel8,
            gidx=pc["gidx"], rel=pc["rel"],
        ))
    return in_maps


# ----------------------------------------------------------------- builder

def build_program(cfg, plan, skip=frozenset()):
    NCH = plan["NCH"]
    NCHA = plan["NCHA"]
    meta = plan["chunk_meta"]
    EPAD = plan["EPAD"]
    W = cfg.W
    GW = cfg.GW

    nc = bacc.Bacc("TRN2", target_bir_lowering=False, debug=False,
                   num_devices=cfg.NC, num_swdge_queues=2)

    # ---- I/O
    t_xt = nc.dram_tensor("xt", [W, 128, cfg.KT, 128], BF16, kind="ExternalInput")
    t_deg = nc.dram_tensor("deg", [cfg.NP], F32, kind="ExternalInput")
    t_w1t = nc.dram_tensor("w1t", [cfg.KT, 128, cfg.H], BF16, kind="ExternalInput")
    t_b1 = nc.dram_tensor("b1rep", [128, cfg.H], F32, kind="ExternalInput")
    t_al = nc.dram_tensor("alrep", [cfg.NL, 128, cfg.H], F32, kind="ExternalInput")
    t_ar = nc.dram_tensor("arrep", [cfg.NL, 128, cfg.H], F32, kind="ExternalInput")
    t_t2 = nc.dram_tensor("t2wt", [cfg.H, cfg.OUT], F32, kind="ExternalInput")
    t_b2 = nc.dram_tensor("b2rep", [128, cfg.OUT], F32, kind="ExternalInput")
    t_iota = nc.dram_tensor("iota", [128, 128], BF16, kind="ExternalInput")
    t_sel8 = nc.dram_tensor("sel8", [8, 8 * 128], BF16, kind="ExternalInput")
    t_gidx = nc.dram_tensor("gidx", [128, EPAD // 16], I16, kind="ExternalInput")
    t_rel = nc.dram_tensor("rel", [128, NCH], F32, kind="ExternalInput")
    t_lsm = nc.dram_tensor("lsm", [cfg.NP, cfg.OUT], F32, kind="ExternalOutput")
    t_emb = nc.dram_tensor("emb", [cfg.NP, cfg.OUT], F32, kind="ExternalOutput")

    # ---- internal DRAM
    TDT = BF16 if cfg.BF16 else F32
    RWE = 256 if cfg.BF16 else 192          # table row elems (512B / 768B)
    d_tab = nc.dram_tensor("tab", [cfg.NP, RWE], TDT)
    d_ar_loc = nc.dram_tensor("ar_loc", [cfg.NP], BF16)
    d_ar_full = nc.dram_tensor("ar_full", [cfg.NPG], BF16, addr_space="Shared")
    # partial sums, partition-major per owner segment: [owner, p, w, c];
    # split into window-halves so RS(A) overlaps half B's edge pass
    d_accA = nc.dram_tensor("accA", [cfg.NC, 128, cfg.WA, cfg.H], BF16)
    d_accB = nc.dram_tensor("accB", [cfg.NC, 128, cfg.WB, cfg.H], BF16)
    d_acc_outA = nc.dram_tensor("acc_outA", [128, cfg.WA, cfg.H], BF16)
    d_acc_outB = nc.dram_tensor("acc_outB", [128, cfg.WB, cfg.H], BF16)

    CS = cfg.CSUP
    rg = [list(range(cfg.NC))]

    with tile.TileContext(nc) as tc:
        with (
            tc.tile_pool(name="const", bufs=1) as cp,
            tc.tile_pool(name="stage", bufs=3) as sp,
            tc.tile_pool(name="gath", bufs=8) as gp,
            tc.tile_pool(name="xload", bufs=2) as xp,
            tc.tile_pool(name="oh", bufs=8) as op,
            tc.tile_pool(name="fl", bufs=4) as fp,
            tc.tile_pool(name="stg2", bufs=2) as s2,
            tc.tile_pool(name="small", bufs=4) as mp,
            tc.tile_pool(name="psum", bufs=2, space="PSUM") as pp,
            tc.tile_pool(name="psbk", bufs=3, space="PSUM") as pb_pool,
            tc.tile_pool(name="psar", bufs=3, space="PSUM") as pa,
        ):
            # ---------- constants / persistent state
            w1 = cp.tile([128, cfg.KT, cfg.H], BF16, tag="w1")
            nc.sync.dma_start(out=w1[:], in_=t_w1t[:].rearrange("k p h -> p k h"))
            b1 = cp.tile([128, cfg.H], F32, tag="b1")
            nc.sync.dma_start(out=b1[:], in_=t_b1[:])
            alr = cp.tile([128, cfg.NL, cfg.H], F32, tag="alr")
            nc.sync.dma_start(out=alr[:], in_=t_al[:].rearrange("l p h -> p l h"))
            arr = cp.tile([128, cfg.NL, cfg.H], F32, tag="arr")
            nc.sync.dma_start(out=arr[:], in_=t_ar[:].rearrange("l p h -> p l h"))
            t2w = cp.tile([cfg.H, cfg.OUT], F32, tag="t2w")
            nc.sync.dma_start(out=t2w[:], in_=t_t2[:])
            b2 = cp.tile([128, cfg.OUT], F32, tag="b2")
            nc.sync.dma_start(out=b2[:], in_=t_b2[:])
            iota = cp.tile([128, 128], BF16, tag="iota")
            nc.sync.dma_start(out=iota[:], in_=t_iota[:])
            gidx = cp.tile([128, EPAD // 16], I16, tag="gidx")
            nc.sync.dma_start(out=gidx[:], in_=t_gidx[:])
            sel8 = cp.tile([8, 8 * 128], BF16, tag="sel8")
            nc.sync.dma_start(out=sel8[:], in_=t_sel8[:])
            rel = cp.tile([128, NCH], F32, tag="rel")
            nc.sync.dma_start(out=rel[:], in_=t_rel[:])
            ident = cp.tile([128, 128], F32, tag="ident")
            make_identity(nc, ident[:])

            h_sb = cp.tile([128, W, cfg.H], F32, tag="h")
            raw_sb = cp.tile([128, W, cfg.H], BF16, tag="raw")
            accl = cp.tile([128, W, cfg.H], BF16, tag="accl")
            dinv = cp.tile([128, W], F32, tag="dinv")
            alc = cp.tile([128, W], F32, tag="alc")
            arc = cp.tile([128, W], F32, tag="arc")
            ar8 = cp.tile([8, W, 128], BF16, tag="ar8")

            # ---------- dinv = (deg>0) / sqrt(max(deg,1))
            degt = mp.tile([128, W], F32, tag="degt")
            with nc.allow_non_contiguous_dma(reason="node-col load"):
                nc.sync.dma_start(out=degt[:], in_=t_deg[:].rearrange("(t p) -> p t", p=128))
            dmax = mp.tile([128, W], F32, tag="dmax")
            nc.vector.tensor_scalar_max(dmax[:], degt[:], 1.0)
            nc.scalar.sqrt(dmax[:], dmax[:])
            nc.vector.reciprocal(dmax[:], dmax[:])
            dnz = mp.tile([128, W], F32, tag="dnz")
            nc.vector.tensor_scalar(dnz[:], degt[:], 0.0, None,
                                    op0=mybir.AluOpType.is_gt)
            nc.vector.tensor_tensor(out=dinv[:], in0=dmax[:], in1=dnz[:],
                                    op=mybir.AluOpType.mult)

            # ---------- phase A: h = relu(x @ t1_w.T + b1)
            AB = 7   # node-tiles per x load
            for t0 in range(0, W if "phasea" not in skip else 0, AB):
                nb = min(AB, W - t0)
                xa = xp.tile([128, AB * cfg.KT * 128], BF16, tag="xa")
                nc.sync.dma_start(
                    out=xa[:, :nb * cfg.KT * 128],
                    in_=t_xt[t0:t0 + nb].rearrange("w p k n -> p w k n"))
                for ti in range(nb):
                    t = t0 + ti
                    ps = pp.tile([128, cfg.H], F32, tag="ps")
                    for k in range(cfg.KT):
                        o = (ti * cfg.KT + k) * 128
                        nc.tensor.matmul(ps[:], lhsT=xa[:, o:o + 128],
                                         rhs=w1[:, k, :],
                                         start=(k == 0), stop=(k == cfg.KT - 1))
                    hb = sp.tile([128, cfg.H], F32, tag="hb")
                    nc.vector.tensor_add(hb[:], ps[:], b1[:])
                    nc.scalar.activation(h_sb[:, t, :], hb[:],
                                         mybir.ActivationFunctionType.Relu)
                    nc.gpsimd.tensor_scalar_mul(raw_sb[:, t, :], h_sb[:, t, :],
                                                EPS)

            # ---------- layers
            FB = cfg.FB
            FL8 = 8   # windows per acc flush DMA (2 psum banks)

            # phase C per-window emission (used early for half A during the
            # last layer's half-B edge pass, late for the rest)
            shs = cp.tile([128, W, cfg.OUT], F32, tag="shs")
            sms = cp.tile([128, W], F32, tag="sms")
            pcst = {"estg": None, "e0": 0}
            pc_done = [False] * W

            def phasec_win(t):
                pc_done[t] = True
                pst = pp.tile([128, 128], F32, tag="ps")
                nc.tensor.transpose(out=pst[:], in_=h_sb[:, t, :],
                                    identity=ident[:])
                ht = sp.tile([128, 128], F32, tag="ht")
                nc.vector.tensor_copy(ht[:], pst[:])
                pse = pp.tile([128, cfg.OUT], F32, tag="ps")
                nc.tensor.matmul(pse[:], lhsT=ht[:], rhs=t2w[:],
                                 start=True, stop=True)
                if pcst["estg"] is None:
                    estg = s2.tile([128, FB, cfg.OUT], F32, tag="estg")
                    pcst["estg"] = estg
                    pcst["e0"] = t
                off = t - pcst["e0"]
                emb = pcst["estg"][:, off, :]
                nc.vector.tensor_add(emb, pse[:], b2[:])
                mx = mp.tile([128, 1], F32, tag="mx")
                nc.vector.tensor_reduce(mx[:], emb, axis=mybir.AxisListType.X,
                                        op=mybir.AluOpType.max)
                nc.vector.tensor_scalar(shs[:, t, :], emb, mx[:], None,
                                        op0=mybir.AluOpType.subtract)
                if off == FB - 1 or t in (cfg.WA - 1, W - 1):
                    nc.sync.dma_start(
                        out=t_emb[(t - off) * 128:(t + 1) * 128, :].rearrange(
                            "(t p) c -> p t c", p=128),
                        in_=pcst["estg"][:, :off + 1, :])
                    pcst["estg"] = None
            for li in range(cfg.NL):
                # node-side: al, ar, hs -> local table
                hstg = None
                for t in range(W if "nprep" not in skip else 0):
                    tmp = sp.tile([128, cfg.H], F32, tag="nprep")
                    nc.vector.scalar_tensor_tensor(
                        tmp[:], h_sb[:, t, :], 1.0, alr[:, li, :],
                        op0=mybir.AluOpType.mult, op1=mybir.AluOpType.mult,
                        accum_out=alc[:, t:t + 1])
                    tmp2 = sp.tile([128, cfg.H], F32, tag="nprep2")
                    nc.vector.scalar_tensor_tensor(
                        tmp2[:], h_sb[:, t, :], 1.0, arr[:, li, :],
                        op0=mybir.AluOpType.mult, op1=mybir.AluOpType.mult,
                        accum_out=arc[:, t:t + 1])
                    if t % FB == 0:
                        hstg = s2.tile([128, FB, cfg.H], TDT, tag="hstg")
                    nc.gpsimd.tensor_scalar_mul(hstg[:, t % FB, :], h_sb[:, t, :],
                                                dinv[:, t:t + 1])
                    if t % FB == FB - 1:
                        t0 = t - FB + 1
                        nc.sync.dma_start(
                            out=d_tab[t0 * 128:(t + 1) * 128, :cfg.H].rearrange(
                                "(t p) c -> p t c", p=128),
                            in_=hstg[:])
                alx = sp.tile([128, W], TDT, tag="alx")
                nc.vector.tensor_copy(alx[:], alc[:])
                with nc.allow_non_contiguous_dma(reason="node-col store"):
                    nc.sync.dma_start(
                        out=d_tab[:, cfg.H:cfg.H + 1].rearrange(
                            "(t p) c -> p (t c)", p=128),
                        in_=alx[:])
                arcb = sp.tile([128, W], BF16, tag="arcb")
                nc.gpsimd.tensor_copy(arcb[:], arc[:])
                with nc.allow_non_contiguous_dma(reason="ar-col store"):
                    nc.sync.dma_start(out=d_ar_loc[:].rearrange("(t p) -> p t", p=128),
                                      in_=arcb[:])
                # edge pass; windows stream in global order, flushed FB at
                # a time into the partition-major partial-sum tensor.  The
                # ar AllGather and RS(A) are emitted a few gather blocks
                # late so their POOL-queue occupancy hides behind gathers
                # that are already in flight.
                psw = None
                psar = None
                fl = None
                rsa_emit = NCHA + 6 * CS      # emit RS(A) after this chunk
                hna_emit = min(NCHA + 20 * CS, NCH - 1)
                nblk = (NCH + CS - 1) // CS
                gtiles = [None] * nblk
                issued = 0
                PF = int(os.environ.get("PF", "7"))  # gather-call prefetch depth

                def issue_gather(k):
                    c0 = k * CS
                    nch_call = min(CS, NCH - c0)
                    ne = nch_call * 128
                    ghs = gp.tile([128, CS * RWE], TDT, tag="ghs")
                    gtiles[k] = ghs
                    if "gather" in skip:
                        nc.vector.memset(ghs[:], 0.0)
                    else:
                        nc.gpsimd.dma_gather(
                            out_ap=ghs[:, :nch_call * RWE].rearrange(
                                "p (c e) -> p c e", e=RWE),
                            in_ap=d_tab[:], idxs_ap=gidx[:, c0 * 8:(c0 + nch_call) * 8],
                            num_idxs=ne, num_idxs_reg=ne, elem_size=RWE,
                            queue_num=0)

                for blk in range(nblk):
                    while issued <= min(blk + PF, nblk - 1):
                        issue_gather(issued)
                        issued += 1
                        if issued == 1:
                            # ar AllGather; owner-major [8, wl, 128] for
                            # SEL8.  Emitted after the first gather call so
                            # that call's DGE isn't queued behind the
                            # collective on POOL.
                            if "ag" not in skip:
                                nc.gpsimd.collective_compute(
                                    "AllGather", mybir.AluOpType.bypass,
                                    replica_groups=rg,
                                    ins=[d_ar_loc[:]], outs=[d_ar_full[:]])
                            nc.sync.dma_start(
                                out=ar8[:],
                                in_=d_ar_full[:].rearrange("(p s n) -> p s n",
                                                           p=8, n=128))
                    c0 = blk * CS
                    nch_call = min(CS, NCH - c0)
                    ghs = gtiles[blk]
                    for j in range(nch_call if "chunk" not in skip else 0):
                        ci = c0 + j
                        ww, first, last = meta[ci]
                        ow_, wl_ = divmod(ww, W)
                        inB = wl_ >= cfg.WA
                        wb_ = wl_ - cfg.WA if inB else wl_
                        nw_ = cfg.WB if inB else cfg.WA
                        sl4 = wb_ % 4
                        if first:
                            # ar_rep[p, n] = ar[window ww, node n]: K=8
                            # one-hot owner-selection matmul from ar8
                            psar = pa.tile([128, 128], F32, tag="psar")
                            nc.tensor.matmul(
                                psar[:], lhsT=sel8[:, ow_ * 128:(ow_ + 1) * 128],
                                rhs=ar8[:, wl_, :],
                                start=True, stop=True)
                            if sl4 == 0:
                                # a psum bank accumulates 4 dst windows
                                pbk = pb_pool.tile([128, 512], F32, tag="pbk")
                            psw = pbk[:, sl4 * 128:(sl4 + 1) * 128]
                        tt = op.tile([128, 128], BF16, tag="tt")
                        nc.scalar.activation(
                            tt[:], psar[:], mybir.ActivationFunctionType.Tanh,
                            bias=ghs[:, j * RWE + cfg.H:j * RWE + cfg.H + 1])
                        ohp = op.tile([128, 128], TDT, tag="ohp")
                        nc.vector.scalar_tensor_tensor(
                            ohp[:], iota[:], rel[:, ci:ci + 1], tt[:],
                            op0=mybir.AluOpType.is_equal,
                            op1=mybir.AluOpType.mult)
                        nc.tensor.matmul(psw, lhsT=ohp[:],
                                         rhs=ghs[:, j * RWE:j * RWE + cfg.H],
                                         start=first, stop=last,
                                         skip_group_check=True)
                        if last:
                            if wb_ % FL8 == 0:
                                fl = fp.tile([128, FL8 * cfg.H], BF16, tag="fl")
                            if sl4 == 3 or wb_ == nw_ - 1:
                                # bank done: one DVE copy moves 4 windows
                                b4 = (wb_ % FL8) - sl4
                                nc.vector.tensor_copy(
                                    fl[:, b4 * cfg.H:(b4 + sl4 + 1) * cfg.H],
                                    pbk[:, :(sl4 + 1) * 128])
                            if wb_ % FL8 == FL8 - 1 or wb_ == nw_ - 1:
                                f0 = wb_ - (wb_ % FL8)
                                n_ = wb_ % FL8 + 1
                                d_ = d_accB if inB else d_accA
                                nc.sync.dma_start(
                                    out=d_[ow_, :, f0:wb_ + 1, :],
                                    in_=fl[:, :n_ * cfg.H])
                        if ci == rsa_emit and "rs" not in skip:
                            # half A was flushed several blocks ago:
                            # reduce-scatter it, hidden behind half B's
                            # in-flight gathers
                            nc.gpsimd.collective_compute(
                                "ReduceScatter", mybir.AluOpType.add,
                                replica_groups=rg,
                                ins=[d_accA[:]], outs=[d_acc_outA[:]])
                            nc.sync.dma_start(out=accl[:, :cfg.WA, :],
                                              in_=d_acc_outA[:])
                        if ci == hna_emit and "rs" not in skip:
                            for t in range(cfg.WA if "nprep" not in skip else 0):
                                nc.vector.scalar_tensor_tensor(
                                    h_sb[:, t, :], accl[:, t, :],
                                    dinv[:, t:t + 1], raw_sb[:, t, :],
                                    op0=mybir.AluOpType.mult,
                                    op1=mybir.AluOpType.add)


                # combine half-B partials; keep own rows
                if "rs" not in skip:
                    nc.gpsimd.collective_compute(
                        "ReduceScatter", mybir.AluOpType.add, replica_groups=rg,
                        ins=[d_accB[:]], outs=[d_acc_outB[:]])
                nc.sync.dma_start(out=accl[:, cfg.WA:, :], in_=d_acc_outB[:])
                # h_new = dinv * acc + raw_eps   (raw_eps = EPS*h0)
                for t in range(cfg.WA, W) if "nprep" not in skip else []:
                    nc.vector.scalar_tensor_tensor(
                        h_sb[:, t, :], accl[:, t, :],
                        dinv[:, t:t + 1], raw_sb[:, t, :],
                        op0=mybir.AluOpType.mult, op1=mybir.AluOpType.add)

            # ---------- phase C tail: remaining windows, then log_softmax
            for t in range(W if "phasec" not in skip else 0):
                if not pc_done[t]:
                    phasec_win(t)
            for t in range(W if "phasec" not in skip else 0):
                ex = sp.tile([128, cfg.OUT], F32, tag="ex")
                nc.scalar.activation(ex[:], shs[:, t, :],
                                     mybir.ActivationFunctionType.Exp)
                nc.vector.tensor_reduce(sms[:, t:t + 1], ex[:],
                                        axis=mybir.AxisListType.X,
                                        op=mybir.AluOpType.add)
            if "phasec" not in skip:
                lns = mp.tile([128, W], F32, tag="lns")
                nc.scalar.activation(lns[:], sms[:],
                                     mybir.ActivationFunctionType.Ln)
                lstg = None
                for t in range(W):
                    if t % FB == 0:
                        lstg = s2.tile([128, FB, cfg.OUT], F32, tag="lstg")
                    nc.vector.tensor_scalar(lstg[:, t % FB, :], shs[:, t, :],
                                            lns[:, t:t + 1],
                                            None, op0=mybir.AluOpType.subtract)
                    if t % FB == FB - 1:
                        nc.sync.dma_start(
                            out=t_lsm[(t - FB + 1) * 128:(t + 1) * 128, :].rearrange(
                                "(t p) c -> p t c", p=128),
                            in_=lstg[:])

    nc.finalize()
    return nc


# ------------------------------------------------------- cached PJRT runner

def _make_runner(nc, n_cores):
    """Like bass2jax.run_bass_via_pjrt, but builds the jitted executable once
    so repeated calls don't re-trace/re-compile."""
    import jax
    import concourse.mybir as mb
    from jax.sharding import Mesh, PartitionSpec
    from jax.experimental.shard_map import shard_map
    from concourse.bass2jax import (install_neuronx_cc_hook, partition_id_tensor,
                                    _bass_exec_p)
    install_neuronx_cc_hook()
    partition_name = nc.partition_id_tensor.name if nc.partition_id_tensor else None
    in_names, out_names, out_avals, zero_outs = [], [], [], []
    for alloc in nc.m.functions[0].allocations:
        if not isinstance(alloc, mb.MemoryLocationSet):
            continue
        name = alloc.memorylocations[0].name
        if alloc.kind == "ExternalInput":
            if name != partition_name:
                in_names.append(name)
        elif alloc.kind == "ExternalOutput":
            out_names.append(name)
            shape = tuple(alloc.tensor_shape)
            dtype = mb.dt.np(alloc.dtype)
            out_avals.append(jax.core.ShapedArray(shape, dtype))
            zero_outs.append(np.zeros(shape, dtype))
    n_params = len(in_names)
    n_outs = len(out_avals)
    all_in_names = list(in_names) + list(out_names)
    if partition_name is not None:
        all_in_names.append(partition_name)
    donate = tuple(range(n_params, n_params + n_outs))

    def _body(*args):
        operands = list(args)
        if partition_name is not None:
            operands.append(partition_id_tensor())
        return tuple(_bass_exec_p.bind(
            *operands, out_avals=tuple(out_avals), in_names=tuple(all_in_names),
            out_names=tuple(out_names), lowering_input_output_aliases=(),
            sim_require_finite=True, sim_require_nnan=True, nc=nc))

    devices = jax.devices()[:n_cores]
    mesh = Mesh(np.asarray(devices), ("core",))
    in_specs = (PartitionSpec("core"),) * (n_params + n_outs)
    out_specs = (PartitionSpec("core"),) * n_outs
    sharded = jax.jit(
        shard_map(_body, mesh=mesh, in_specs=in_specs, out_specs=out_specs,
                  check_rep=False),
        donate_argnums=donate, keep_unused=True)

    def call(in_maps):
        concat_in = [
            np.concatenate([np.asarray(in_maps[c][k]) for c in range(n_cores)], 0)
            for k in in_names
        ]
        concat_zeros = [
            np.zeros((n_cores * z.shape[0], *z.shape[1:]), z.dtype)
            for z in zero_outs
        ]
        out_arrs = sharded(*concat_in, *concat_zeros)
        jax.block_until_ready(out_arrs)
        return [
            {k: np.asarray(out_arrs[i]).reshape(n_cores, *out_avals[i].shape)[c]
             for i, k in enumerate(out_names)}
            for c in range(n_cores)
        ]

    return call


# The previous pull-based kernel measured 2127000 ns on hardware (layer
# differencing) and simulates at 1738.6 us under the CoreSim cost model;
# this push kernel simulates at 1084.4 us.  Scaling the hardware estimate
# by the simulated ratio: 2127000 * 1084.4/1738.6.
HW_EXEC_NS_ESTIMATE = 1327000

# ----------------------------------------------------------------- entry

_CACHE = {}


def run(cfg, inputs, trace=False):
    ei = np.asarray(inputs["edge_index"])
    key = (cfg.N, cfg.E, cfg.NL, hash(ei.tobytes()))
    if key in _CACHE:
        runner, plan = _CACHE[key]
    else:
        plan = plan_edges(cfg, ei)
        nc = build_program(cfg, plan)
        runner = _make_runner(nc, cfg.NC)
        _CACHE[key] = (runner, plan)
    in_maps = shard_inputs(cfg, inputs, plan)
    results = runner(in_maps)
    lsm = np.concatenate([results[c]["lsm"][:cfg.NV] for c in range(cfg.NC)], 0)
    emb = np.concatenate([results[c]["emb"][:cfg.NV] for c in range(cfg.NC)], 0)
    return (lsm, emb), None


def kernel(**inputs):
    (lsm, emb), _ = run(FULL, inputs)
    return lsm, emb
